# revision 28
# baseline (speedup 1.0000x reference)
"""Block-sparse (block-diagonal local) attention head for Trainium2, 8-way
data-parallel over the batch dimension (one batch element per NeuronCore).

Contract: kernel(**inputs) takes the FULL inputs from setup_inputs() and
returns the FULL output of reference(): out [8, 4096, 128] float32.

Per-core math (batch b):
  qT = (x_b @ Wq)^T, kT = (x_b @ Wk)^T, vT = (x_b @ Wv)^T   (Dh on partitions)
  per 128-token block j:
    v_j   = transpose(vT_j)                    (PE transpose, token-major)
    sT_j  = scoresT[k,q] = sum_d kT[d,k] qT[d,q]
    PT_j  = exp(sT_j / sqrt(Dh))               (no max-subtraction; logits are
                                                O(10) here, softmax algebra is
                                                exact without it)
    o'_j  = PT_j^T @ [v_j | 1 | 1]             (ones columns give row sums;
                                                two of them because f32r
                                                matmuls need an even free dim)
    out_j = o'_j[:, :128] * (1 / o'_j[:, 128])

Dtype strategy (MM_MODE):
  "bf16": projections in bf16 (1 cyc/row on the PE), attention chain in
          float32r (single-pass fp32_mode=HIGH, ~13-14 mantissa bits).
  "f32r": everything float32r (2 cyc/row projections).
  "f32":  everything fp32 (2-pass matmuls, slowest, exact).

Host-side prep (sharding freedom): batch b -> core b; x[b] is transposed and
repacked to xp [128, KC, S] (partition-major, so each DMA reads long
contiguous runs); the three projection weights are repacked into one
wp [128, 3, KC, Dh] tensor; the PE-transpose identity ships from the host
(affine_select cannot legally produce f32r data).
"""

import numpy as np
from contextlib import ExitStack

B, S, D, Dh, BLOCK = 8, 4096, 1024, 128, 128
KC = D // 128  # contraction chunks of 128
MT = 512       # token tile (moving free dim of projection matmuls)
STS = 512      # token super-tile per x DMA
NST = S // STS
JT = MT // BLOCK
SCALE = float(1.0 / np.sqrt(np.float32(Dh)))

MM_MODE = "bf16"

_CACHE = {}


def _build(mode):
    import concourse.bass as bass
    import concourse.mybir as mybir
    import concourse.tile as tile
    from concourse import bacc

    f32 = mybir.dt.float32
    f32r = mybir.dt.float32r
    bf16 = mybir.dt.bfloat16
    ts = bass.ts

    # dtype of the x/W data path feeding the projection matmuls
    proj_dt = {"f32": f32, "f32r": f32r, "bf16": bf16}[mode]
    # dtype of the scores chain (qT/kT staging)
    attn_dt = f32 if mode == "f32" else f32r
    # dtype of the v/transpose/o' chain (P, v, identity): bf16 rounding of
    # P (in [0,1]) and v adds nothing over the bf16 projection error
    trans_dt = bf16 if mode == "bf16" else attn_dt

    nc = bacc.Bacc("TRN2", target_bir_lowering=False, debug=False)

    xp = nc.dram_tensor("xp", [128, KC, S], proj_dt, kind="ExternalInput").ap()
    ident_d = nc.dram_tensor("ident", [128, 128], trans_dt, kind="ExternalInput").ap()
    wp = nc.dram_tensor("wp", [128, 3, KC, Dh], proj_dt, kind="ExternalInput").ap()
    out = nc.dram_tensor("out", [S, Dh], f32, kind="ExternalOutput").ap()

    with tile.TileContext(nc) as tc, ExitStack() as ctx:
        wpool = ctx.enter_context(tc.tile_pool(name="w", bufs=1))
        cpool = ctx.enter_context(tc.tile_pool(name="const", bufs=1))
        xpool = ctx.enter_context(tc.tile_pool(name="x", bufs=3))
        spool = ctx.enter_context(tc.tile_pool(name="s", bufs=2))
        apool = ctx.enter_context(tc.tile_pool(name="a", bufs=4))
        opool = ctx.enter_context(tc.tile_pool(name="o", bufs=2))
        ppool = ctx.enter_context(tc.tile_pool(name="proj_ps", bufs=3, space="PSUM"))
        qpool = ctx.enter_context(tc.tile_pool(name="attn_ps", bufs=5, space="PSUM"))

        # Two HWDGE rings (sync=SP, scalar=ACT). Startup order is chosen so
        # the first projection matmuls are gated only by small transfers:
        # scalar: wq -> x chunks 4..7 -> ident; sync: x chunk 0 -> 1..3 -> wk|wv
        wp_t = wpool.tile([128, 3, KC, Dh], proj_dt, tag="wp")
        # wq chunk 0 alone gates the first matmul: load it first and tiny
        nc.scalar.dma_start(wp_t[:, 0:1, 0:1], wp[:, 0:1, 0:1])
        nc.scalar.dma_start(wp_t[:, 0:1, 1:KC], wp[:, 0:1, 1:KC])
        ident = cpool.tile([128, 128], trans_dt, tag="ident")

        for st in range(NST):
            s0 = st * STS
            # x super-tile [128, KC, STS]; first one split in two so the
            # first projection matmuls start after half a super-tile
            xt = xpool.tile([128, KC, STS], proj_dt, tag="xt")
            if st == 0:
                nc.sync.dma_start(xt[:, 0:1], xp[:, 0:1, s0 : s0 + STS])
                nc.sync.dma_start(xt[:, 1:4], xp[:, 1:4, s0 : s0 + STS])
                nc.scalar.dma_start(xt[:, 4:KC], xp[:, 4:KC, s0 : s0 + STS])
                nc.sync.dma_start(wp_t[:, 1:2], wp[:, 1:2])  # wk
                nc.scalar.dma_start(wp_t[:, 2:3], wp[:, 2:3])  # wv
                nc.scalar.dma_start(ident[:], ident_d[:])
            else:
                # split every super-tile across both rings to halve latency
                nc.sync.dma_start(xt[:, 0:4], xp[:, 0:4, s0 : s0 + STS])
                nc.scalar.dma_start(xt[:, 4:KC], xp[:, 4:KC, s0 : s0 + STS])

            for sub in range(STS // MT):
                moff = sub * MT
                m0 = s0 + moff

                # Projections (Dh on partitions):
                # pT[d, m] = sum_k W[k, d] xT[k, m]
                pT_sbs = []
                for wi, tag, copy_eng, sb_dt in (
                    (0, "qT", nc.vector, attn_dt),
                    (1, "kT", nc.scalar, attn_dt),
                    (2, "vT", nc.vector, trans_dt),
                ):
                    pT_ps = ppool.tile([128, MT], f32, tag="proj")
                    for k in range(KC):
                        nc.tensor.matmul(
                            pT_ps[:],
                            wp_t[:, wi, k, :],
                            xt[:, k, moff : moff + MT],
                            start=(k == 0),
                            stop=(k == KC - 1),
                        )
                    pT_sb = spool.tile([128, MT], sb_dt, tag=tag)
                    if copy_eng is nc.scalar:
                        nc.scalar.copy(pT_sb[:], pT_ps[:])
                    else:
                        nc.vector.tensor_copy(pT_sb[:], pT_ps[:])
                    pT_sbs.append(pT_sb)
                qT_sb, kT_sb, vT_sb = pT_sbs

                # Attention on the JT blocks of this m-tile. Output blocks
                # collect into one [128, JT*BLOCK] tile -> one DMA per m-tile.
                o_mt = opool.tile([128, JT, BLOCK], f32, tag="o_mt")

                # All JT blocks' scoresT into one PSUM bank (disjoint column
                # slices of one accumulation group), then a single exp.
                sT_big = qpool.tile([128, JT * BLOCK], f32, tag="attn")
                for j in range(JT):
                    blk = ts(j, BLOCK)
                    nc.tensor.matmul(
                        sT_big[:, blk],
                        kT_sb[:, blk],
                        qT_sb[:, blk],
                        start=(j == 0),
                        stop=(j == JT - 1),
                    )
                PT_big = apool.tile([128, JT * BLOCK], trans_dt, tag="PT")
                nc.scalar.activation(
                    PT_big[:], sT_big[:], mybir.ActivationFunctionType.Exp, scale=SCALE
                )

                for j in range(JT):
                    blk = ts(j, BLOCK)
                    # token-major v block via PE transpose
                    v_ps = qpool.tile([128, BLOCK], trans_dt, tag="attn")
                    nc.tensor.transpose(v_ps[:], vT_sb[:, blk], ident[:])
                    # f32r matmuls need an even moving free dim:
                    # two ones columns
                    v_sb = apool.tile([128, BLOCK + 2], trans_dt, tag="v")
                    nc.vector.tensor_copy(v_sb[:, 0:BLOCK], v_ps[:])
                    ones_col = v_sb[:, BLOCK : BLOCK + 2]
                    nc.vector.memset(
                        ones_col.bitcast(f32) if trans_dt == f32r else ones_col,
                        1.0,
                    )

                    # o'[q, :Dh] = unnormalized attn out; o'[q, Dh] = row sum
                    o_ps = qpool.tile([128, BLOCK + 2], f32, tag="attn")
                    nc.tensor.matmul(
                        o_ps[:], PT_big[:, blk], v_sb[:], start=True, stop=True
                    )

                    r_sb = apool.tile([128, 1], f32, tag="r")
                    nc.vector.reciprocal(r_sb[:], o_ps[:, BLOCK : BLOCK + 1])
                    # normalize on ACT (Copy with per-partition scale) to keep
                    # the DVE off the block-chain critical path
                    nc.scalar.activation(
                        o_mt[:, j, :],
                        o_ps[:, 0:BLOCK],
                        mybir.ActivationFunctionType.Copy,
                        scale=r_sb[:],
                    )

                # out[m0 + c*BLOCK + p, d] <- o_mt[p, c, d]
                out_view = out[m0 : m0 + MT, :].rearrange(
                    "(c p) d -> p c d", p=BLOCK
                )
                if st == NST - 1 and sub == STS // MT - 1:
                    # last m-tile: store per block so the final transfer is
                    # small and the end-of-kernel drain starts sooner
                    for j in range(JT):
                        nc.sync.dma_start(out_view[:, j : j + 1, :], o_mt[:, j : j + 1, :])
                else:
                    nc.sync.dma_start(out_view, o_mt[:])

    nc.compile()
    return nc


def _get_nc():
    if MM_MODE not in _CACHE:
        _CACHE[MM_MODE] = _build(MM_MODE)
    return _CACHE[MM_MODE]


def _casts():
    if MM_MODE == "bf16":
        import ml_dtypes

        proj_np = ml_dtypes.bfloat16
    else:
        proj_np = np.float32
    return proj_np


def make_in_maps(x, Wq, Wk, Wv):
    proj_np = _casts()
    # wp[p, i, k, d] = W_i[k*128 + p, d]
    wp = np.stack(
        [np.asarray(w).reshape(KC, 128, Dh).transpose(1, 0, 2) for w in (Wq, Wk, Wv)],
        axis=1,
    )
    wp_h = np.ascontiguousarray(wp.astype(proj_np))
    ident_h = np.eye(128, dtype=proj_np if MM_MODE == "bf16" else np.float32)
    x = np.asarray(x)
    maps = []
    for b in range(B):
        # xp[p, k, s] = x[b].T[k*128 + p, s]
        xp = np.asarray(x[b], dtype=proj_np).T.reshape(KC, 128, S).transpose(1, 0, 2)
        maps.append(
            {
                "xp": np.ascontiguousarray(xp),
                "wp": wp_h,
                "ident": ident_h,
            }
        )
    return maps


def kernel(x, Wq, Wk, Wv):
    from concourse.bass_utils import run_bass_kernel_spmd

    nc = _get_nc()
    in_maps = make_in_maps(x, Wq, Wk, Wv)
    res = run_bass_kernel_spmd(nc, in_maps, list(range(B))).results
    return np.stack([res[b]["out"] for b in range(B)], axis=0)


# revision 29
# speedup vs baseline: 1.0306x; 1.0306x over previous
"""Block-sparse (block-diagonal local) attention head for Trainium2, 8-way
data-parallel over the batch dimension (one batch element per NeuronCore).

Contract: kernel(**inputs) takes the FULL inputs from setup_inputs() and
returns the FULL output of reference(): out [8, 4096, 128] float32.

Per-core math (batch b):
  qT = (x_b @ Wq)^T, kT = (x_b @ Wk)^T, vT = (x_b @ Wv)^T   (Dh on partitions)
  per 128-token block j:
    v_j   = transpose(vT_j)                    (PE transpose, token-major)
    sT_j  = scoresT[k,q] = sum_d kT[d,k] qT[d,q]
    PT_j  = exp(sT_j / sqrt(Dh))               (no max-subtraction; logits are
                                                O(10) here, softmax algebra is
                                                exact without it)
    o'_j  = PT_j^T @ [v_j | 1 | 1]             (ones columns give row sums;
                                                two of them because f32r
                                                matmuls need an even free dim)
    out_j = o'_j[:, :128] * (1 / o'_j[:, 128])

Dtype strategy (MM_MODE):
  "bf16": projections in bf16 (1 cyc/row on the PE), attention chain in
          float32r (single-pass fp32_mode=HIGH, ~13-14 mantissa bits).
  "f32r": everything float32r (2 cyc/row projections).
  "f32":  everything fp32 (2-pass matmuls, slowest, exact).

Host-side prep (sharding freedom): batch b -> core b; x[b] is transposed and
repacked to xp [128, KC, S] (partition-major, so each DMA reads long
contiguous runs); the three projection weights are repacked into one
wp [128, 3, KC, Dh] tensor; the PE-transpose identity ships from the host
(affine_select cannot legally produce f32r data).
"""

import numpy as np
from contextlib import ExitStack

B, S, D, Dh, BLOCK = 8, 4096, 1024, 128, 128
KC = D // 128  # contraction chunks of 128
MT = 512       # token tile (moving free dim of projection matmuls)
STS = 512      # token super-tile per x DMA
NST = S // STS
JT = MT // BLOCK
SCALE = float(1.0 / np.sqrt(np.float32(Dh)))

MM_MODE = "bf16"

_CACHE = {}


def _build(mode):
    import concourse.bass as bass
    import concourse.mybir as mybir
    import concourse.tile as tile
    from concourse import bacc

    f32 = mybir.dt.float32
    f32r = mybir.dt.float32r
    bf16 = mybir.dt.bfloat16
    ts = bass.ts

    # dtype of the x/W data path feeding the projection matmuls
    proj_dt = {"f32": f32, "f32r": f32r, "bf16": bf16}[mode]
    # dtype of the scores chain (qT/kT staging)
    attn_dt = f32 if mode == "f32" else f32r
    # dtype of the v/transpose/o' chain (P, v, identity): bf16 rounding of
    # P (in [0,1]) and v adds nothing over the bf16 projection error
    trans_dt = bf16 if mode == "bf16" else attn_dt

    nc = bacc.Bacc("TRN2", target_bir_lowering=False, debug=False)

    xp = nc.dram_tensor("xp", [128, KC, S], proj_dt, kind="ExternalInput").ap()
    ident_d = nc.dram_tensor("ident", [128, 128], trans_dt, kind="ExternalInput").ap()
    wp = nc.dram_tensor("wp", [128, 3, KC, Dh], proj_dt, kind="ExternalInput").ap()
    out = nc.dram_tensor("out", [S, Dh], f32, kind="ExternalOutput").ap()

    with tile.TileContext(nc) as tc, ExitStack() as ctx:
        wpool = ctx.enter_context(tc.tile_pool(name="w", bufs=1))
        cpool = ctx.enter_context(tc.tile_pool(name="const", bufs=1))
        xpool = ctx.enter_context(tc.tile_pool(name="x", bufs=3))
        spool = ctx.enter_context(tc.tile_pool(name="s", bufs=2))
        apool = ctx.enter_context(tc.tile_pool(name="a", bufs=4))
        opool = ctx.enter_context(tc.tile_pool(name="o", bufs=2))
        ppool = ctx.enter_context(tc.tile_pool(name="proj_ps", bufs=3, space="PSUM"))
        qpool = ctx.enter_context(tc.tile_pool(name="attn_ps", bufs=5, space="PSUM"))

        # Two HWDGE rings (sync=SP, scalar=ACT). Startup order is chosen so
        # the first projection matmuls are gated only by small transfers:
        # scalar: wq -> x chunks 4..7 -> ident; sync: x chunk 0 -> 1..3 -> wk|wv
        wp_t = wpool.tile([128, 3, KC, Dh], proj_dt, tag="wp")
        # wq chunk 0 alone gates the first matmul: load it first and tiny
        nc.scalar.dma_start(wp_t[:, 0:1, 0:1], wp[:, 0:1, 0:1])
        nc.scalar.dma_start(wp_t[:, 0:1, 1:KC], wp[:, 0:1, 1:KC])
        ident = cpool.tile([128, 128], trans_dt, tag="ident")

        for st in range(NST):
            s0 = st * STS
            # x super-tile [128, KC, STS]; first one split in two so the
            # first projection matmuls start after half a super-tile
            xt = xpool.tile([128, KC, STS], proj_dt, tag="xt")
            if st == 0:
                nc.sync.dma_start(xt[:, 0:1], xp[:, 0:1, s0 : s0 + STS])
                nc.sync.dma_start(xt[:, 1:4], xp[:, 1:4, s0 : s0 + STS])
                nc.scalar.dma_start(xt[:, 4:KC], xp[:, 4:KC, s0 : s0 + STS])
                nc.sync.dma_start(wp_t[:, 1:2], wp[:, 1:2])  # wk
                nc.scalar.dma_start(wp_t[:, 2:3], wp[:, 2:3])  # wv
                nc.scalar.dma_start(ident[:], ident_d[:])
            else:
                # split every super-tile across both rings to halve latency
                nc.sync.dma_start(xt[:, 0:4], xp[:, 0:4, s0 : s0 + STS])
                nc.scalar.dma_start(xt[:, 4:KC], xp[:, 4:KC, s0 : s0 + STS])

            for sub in range(STS // MT):
                moff = sub * MT
                m0 = s0 + moff

                # Projections (Dh on partitions):
                # pT[d, m] = sum_k W[k, d] xT[k, m]
                pT_sbs = []
                for wi, tag, copy_eng, sb_dt in (
                    (0, "qT", nc.vector, attn_dt),
                    (1, "kT", nc.scalar, attn_dt),
                    (2, "vT", nc.vector, trans_dt),
                ):
                    pT_ps = ppool.tile([128, MT], f32, tag="proj")
                    for k in range(KC):
                        nc.tensor.matmul(
                            pT_ps[:],
                            wp_t[:, wi, k, :],
                            xt[:, k, moff : moff + MT],
                            start=(k == 0),
                            stop=(k == KC - 1),
                        )
                    pT_sb = spool.tile([128, MT], sb_dt, tag=tag)
                    if copy_eng is nc.scalar:
                        nc.scalar.copy(pT_sb[:], pT_ps[:])
                    else:
                        nc.vector.tensor_copy(pT_sb[:], pT_ps[:])
                    pT_sbs.append(pT_sb)
                qT_sb, kT_sb, vT_sb = pT_sbs

                # Attention on the JT blocks of this m-tile. Output blocks
                # collect into one [128, JT*BLOCK] tile -> one DMA per m-tile.
                o_mt = opool.tile([128, JT, BLOCK], f32, tag="o_mt")

                # v transposes first: their PSUM->SBUF copies overlap the
                # scores matmuls and the exp, so the o' matmuls never wait
                v_sbs = []
                for j in range(JT):
                    blk = ts(j, BLOCK)
                    v_ps = qpool.tile([128, BLOCK], trans_dt, tag="attn")
                    nc.tensor.transpose(v_ps[:], vT_sb[:, blk], ident[:])
                    # f32r matmuls need an even moving free dim:
                    # two ones columns
                    v_sb = apool.tile([128, BLOCK + 2], trans_dt, tag="v")
                    nc.vector.tensor_copy(v_sb[:, 0:BLOCK], v_ps[:])
                    ones_col = v_sb[:, BLOCK : BLOCK + 2]
                    nc.vector.memset(
                        ones_col.bitcast(f32) if trans_dt == f32r else ones_col,
                        1.0,
                    )
                    v_sbs.append(v_sb)

                # All JT blocks' scoresT into one PSUM bank (disjoint column
                # slices of one accumulation group), then a single exp.
                sT_big = qpool.tile([128, JT * BLOCK], f32, tag="attn")
                for j in range(JT):
                    blk = ts(j, BLOCK)
                    nc.tensor.matmul(
                        sT_big[:, blk],
                        kT_sb[:, blk],
                        qT_sb[:, blk],
                        start=(j == 0),
                        stop=(j == JT - 1),
                    )
                PT_big = apool.tile([128, JT * BLOCK], trans_dt, tag="PT")
                nc.scalar.activation(
                    PT_big[:], sT_big[:], mybir.ActivationFunctionType.Exp, scale=SCALE
                )

                for j in range(JT):
                    blk = ts(j, BLOCK)
                    v_sb = v_sbs[j]
                    # o'[q, :Dh] = unnormalized attn out; o'[q, Dh] = row sum
                    o_ps = qpool.tile([128, BLOCK + 2], f32, tag="attn")
                    nc.tensor.matmul(
                        o_ps[:], PT_big[:, blk], v_sb[:], start=True, stop=True
                    )

                    r_sb = apool.tile([128, 1], f32, tag="r")
                    nc.vector.reciprocal(r_sb[:], o_ps[:, BLOCK : BLOCK + 1])
                    # normalize on ACT (Copy with per-partition scale) to keep
                    # the DVE off the block-chain critical path
                    nc.scalar.activation(
                        o_mt[:, j, :],
                        o_ps[:, 0:BLOCK],
                        mybir.ActivationFunctionType.Copy,
                        scale=r_sb[:],
                    )

                # out[m0 + c*BLOCK + p, d] <- o_mt[p, c, d]
                out_view = out[m0 : m0 + MT, :].rearrange(
                    "(c p) d -> p c d", p=BLOCK
                )
                if st == NST - 1 and sub == STS // MT - 1:
                    # last m-tile: store per block so the final transfer is
                    # small and the end-of-kernel drain starts sooner
                    for j in range(JT):
                        nc.sync.dma_start(out_view[:, j : j + 1, :], o_mt[:, j : j + 1, :])
                else:
                    nc.sync.dma_start(out_view, o_mt[:])

    nc.compile()
    return nc


def _get_nc():
    if MM_MODE not in _CACHE:
        _CACHE[MM_MODE] = _build(MM_MODE)
    return _CACHE[MM_MODE]


def _casts():
    if MM_MODE == "bf16":
        import ml_dtypes

        proj_np = ml_dtypes.bfloat16
    else:
        proj_np = np.float32
    return proj_np


def make_in_maps(x, Wq, Wk, Wv):
    proj_np = _casts()
    # wp[p, i, k, d] = W_i[k*128 + p, d]
    wp = np.stack(
        [np.asarray(w).reshape(KC, 128, Dh).transpose(1, 0, 2) for w in (Wq, Wk, Wv)],
        axis=1,
    )
    wp_h = np.ascontiguousarray(wp.astype(proj_np))
    ident_h = np.eye(128, dtype=proj_np if MM_MODE == "bf16" else np.float32)
    x = np.asarray(x)
    maps = []
    for b in range(B):
        # xp[p, k, s] = x[b].T[k*128 + p, s]
        xp = np.asarray(x[b], dtype=proj_np).T.reshape(KC, 128, S).transpose(1, 0, 2)
        maps.append(
            {
                "xp": np.ascontiguousarray(xp),
                "wp": wp_h,
                "ident": ident_h,
            }
        )
    return maps


def kernel(x, Wq, Wk, Wv):
    from concourse.bass_utils import run_bass_kernel_spmd

    nc = _get_nc()
    in_maps = make_in_maps(x, Wq, Wk, Wv)
    res = run_bass_kernel_spmd(nc, in_maps, list(range(B))).results
    return np.stack([res[b]["out"] for b in range(B)], axis=0)
